# revision 1
# baseline (speedup 1.0000x reference)
"""Trainium2 Bass kernel for nn_CausalConvolution (dense_cnn).

Reference computation (B=4, S=4096, H=2048, CIN=COUT=4096, K=4, G=8):
    h   = x @ W_in.T + b_in                       # [B,S,CIN]
    y   = silu(causal_grouped_conv1d(h) + conv_b) # [B,S,COUT], groups=8, k=4
    out = y @ W_out.T + b_out                     # [B,S,H]

Sharding: one conv group per NeuronCore (G = 8 = n_cores).
Core g computes channels [g*512, (g+1)*512) of h (column-parallel W_in),
its conv group (512 in / 512 out channels), and a row-parallel partial of
the output projection. Host sums the 8 partials and adds b_out. No
cross-core communication on device.

All matmuls run in bf16 (fp32 PSUM accumulation); everything is kept in
"transposed" [channel, time] layout on-chip so the contraction dim always
sits on SBUF partitions without any on-chip transposes.

Schedule notes: PE is the bottleneck (6144 N=512 matmuls/core ~= 1.31 ms
at the bf16 streaming limit), so the kernel front-loads only the DMAs the
first matmuls need (w_in + first x tile), gates the conv/out weights
behind them, pre-warms the PE clock (HAM) with scratch matmuls during the
initial DMA wait, and runs stage 1 one time-tile ahead of stages 2/3.
"""

import numpy as np
import ml_dtypes

# Problem constants (hardcoded per the harness contract).
B, S, H = 4, 4096, 2048
CIN = COUT = 4096
KT = 4          # conv taps
G = 8           # conv groups == number of cores
CG = CIN // G   # 512 channels per group/core
T = B * S       # 16384 flattened time steps
NCORES = 8

HK = H // 128       # 16 contraction chunks for stage 1
CT = CG // 128      # 4 chunks of the per-core channel dim
TTILE = 512         # time-tile (N of every matmul)
NH = H // TTILE     # 4 output-column chunks of stage 3

_BF16 = ml_dtypes.bfloat16

_CACHE = {}

# test.py introspection: the most recent BassKernelResults from a run.
LAST_RESULTS = None


def _build_nc():
    import concourse.bass as bass
    import concourse.mybir as mybir
    import concourse.tile as tile
    from concourse.tile import add_dep_helper
    from concourse import bacc

    dt = mybir.dt
    AF = mybir.ActivationFunctionType

    nc = bacc.Bacc(
        "TRN2", target_bir_lowering=False, debug=False, num_devices=NCORES
    )

    xT = nc.dram_tensor("xT", [128, HK, T], dt.bfloat16, kind="ExternalInput")
    w_in = nc.dram_tensor("w_in", [128, CT, HK, 128], dt.bfloat16, kind="ExternalInput")
    cw = nc.dram_tensor("cw", [128, KT, CT, CG], dt.bfloat16, kind="ExternalInput")
    wo = nc.dram_tensor("wo", [128, CT, H], dt.bfloat16, kind="ExternalInput")
    b_in = nc.dram_tensor("b_in", [128, CT], dt.float32, kind="ExternalInput")
    cb = nc.dram_tensor("cb", [128, CT], dt.float32, kind="ExternalInput")
    out = nc.dram_tensor("out", [T, H], dt.float32, kind="ExternalOutput")

    n_tt = S // TTILE  # time tiles per batch

    with tile.TileContext(nc) as tc:
        # PE warmup: dep-free matmuls on scratch data run while the first
        # weight/x DMAs are in flight, so HAM un-throttles (K=8/8) before
        # the real matmul stream begins.
        with (
            tc.tile_pool(name="warm", bufs=1) as warmpool,
            tc.tile_pool(name="warmps", bufs=1, space="PSUM") as warmpspool,
        ):
            scratch = warmpool.tile([128, 640], dt.bfloat16)
            nc.vector.memset(scratch[:], 0.0)
            wps = warmpspool.tile([128, TTILE], dt.float32)
            for _ in range(22):
                nc.tensor.matmul(
                    wps[:], scratch[:, 0:128], scratch[:, 128:640],
                    start=True, stop=True,
                )
        with (
            tc.tile_pool(name="weights", bufs=1) as wpool,
            tc.tile_pool(name="xin", bufs=3) as xpool,
            tc.tile_pool(name="hbuf", bufs=2) as hpool,
            tc.tile_pool(name="ybuf", bufs=3) as ypool,
            tc.tile_pool(name="obuf", bufs=2) as opool,
            tc.tile_pool(name="ps1", bufs=2, space="PSUM") as ps1pool,
            tc.tile_pool(name="ps2", bufs=2, space="PSUM") as ps2pool,
            tc.tile_pool(name="ps3", bufs=4, space="PSUM") as ps3pool,
        ):
            # Startup DMA scheduling. Two facts drive the shape: (1) a
            # single dma_start descriptor streams on one DMA engine at
            # only ~65 GB/s, so anything urgent must be SPLIT into
            # several descriptors for parallel engine pickup; (2) all
            # in-flight descriptors share HBM bandwidth fairly, so bulk
            # loads must be GATED behind the urgent ones or everything
            # finishes late together.
            # Phase A0 (ungated): stage-1 tile-0's c=0 weights + full
            # first x tile, as 12 parallel descriptors. Later phases are
            # ordered by consumption deadline: c=1 lands ~1.5 µs after
            # A0, c=2/3 after that — each just ahead of stage 1's use.
            w_in_sb = wpool.tile([128, CT, HK, 128], dt.bfloat16)
            xt_first = xpool.tile([128, HK, TTILE], dt.bfloat16, tag="xt")
            for half in range(2):
                a0d = nc.sync.dma_start(
                    w_in_sb[:, 0, 8 * half : 8 * half + 8, :],
                    w_in[:, 0, 8 * half : 8 * half + 8, :],
                )
            for q in range(8):
                a0d = nc.sync.dma_start(
                    xt_first[:, 2 * q : 2 * q + 2, :],
                    xT[:, 2 * q : 2 * q + 2, 0:TTILE],
                )
            # Phase A1a (gated on A0): c=1 weights only.
            for half in range(2):
                a1ad = nc.sync.dma_start(
                    w_in_sb[:, 1, 8 * half : 8 * half + 8, :],
                    w_in[:, 1, 8 * half : 8 * half + 8, :],
                )
                add_dep_helper(a1ad.ins, a0d.ins, reason="phase A1a")
            # Phase A1b (gated on A1a): the rest of stage-1's weights.
            bin_sb = wpool.tile([128, CT], dt.float32)
            bd = nc.sync.dma_start(bin_sb[:], b_in[:])
            add_dep_helper(bd.ins, a1ad.ins, reason="phase A1b")
            for cc in range(2, CT):
                for half in range(2):
                    a1d = nc.sync.dma_start(
                        w_in_sb[:, cc, 8 * half : 8 * half + 8, :],
                        w_in[:, cc, 8 * half : 8 * half + 8, :],
                    )
                    add_dep_helper(a1d.ins, a1ad.ins, reason="phase A1b")
            cb_sb = wpool.tile([128, CT], dt.float32)
            cbd = nc.sync.dma_start(cb_sb[:], cb[:])
            add_dep_helper(cbd.ins, a0d.ins, reason="phase A1")
            # Bulk weights, deferred further (needed only after one /
            # two full stage-1 tiles respectively), 4 descriptors each.
            cw_sb = wpool.tile([128, KT, CT, CG], dt.bfloat16)
            for k in range(KT):
                cwd = nc.sync.dma_start(cw_sb[:, k], cw[:, k])
                add_dep_helper(cwd.ins, a1d.ins, reason="defer conv weights")
            wo_sb = wpool.tile([128, CT, H], dt.bfloat16)
            for oo in range(CT):
                wod = nc.sync.dma_start(wo_sb[:, oo], wo[:, oo])
                add_dep_helper(wod.ins, cwd.ins, reason="defer out weights")

            tiles = [(b, tt) for b in range(B) for tt in range(n_tt)]
            hts = {}   # batch -> hT tile
            yts = {}   # (b, tt) -> y tile

            def stage1(b, tt):
                t0 = tt * TTILE
                tg = b * S + t0
                if tt == 0:
                    # h^T for this batch: [c, t] with a 3-column zero halo
                    # in front so causal taps at batch start read zeros.
                    hts[b] = hpool.tile(
                        [128, CT, KT - 1 + S], dt.bfloat16, tag="hT", name="hT"
                    )
                    nc.vector.memset(hts[b][:, :, 0 : KT - 1], 0.0)
                hT = hts[b]
                if b == 0 and tt == 0:
                    xt = xt_first
                else:
                    xt = xpool.tile([128, HK, TTILE], dt.bfloat16, tag="xt")
                    nc.sync.dma_start(xt[:, 0:8, :], xT[:, 0:8, tg : tg + TTILE])
                    nc.sync.dma_start(xt[:, 8:16, :], xT[:, 8:16, tg : tg + TTILE])
                for c in range(CT):
                    ps = ps1pool.tile([128, TTILE], dt.float32)
                    for hk in range(HK):
                        nc.tensor.matmul(
                            ps[:],
                            w_in_sb[:, c, hk, :],
                            xt[:, hk, :],
                            start=(hk == 0),
                            stop=(hk == HK - 1),
                        )
                    nc.scalar.activation(
                        hT[:, c, KT - 1 + t0 : KT - 1 + t0 + TTILE],
                        ps[:],
                        AF.Identity,
                        bias=bin_sb[:, c : c + 1],
                    )

            def stage23(b, tt):
                t0 = tt * TTILE
                tg = b * S + t0
                hT = hts[b]
                # Stage 2: causal grouped conv as 16 accumulated matmuls
                yt = ypool.tile([128, CT, TTILE], dt.bfloat16, tag="yt")
                for o in range(CT):
                    ps = ps2pool.tile([128, TTILE], dt.float32)
                    n_acc = KT * CT
                    acc = 0
                    for ik in range(CT):
                        for k in range(KT):
                            nc.tensor.matmul(
                                ps[:],
                                cw_sb[:, k, ik, o * 128 : (o + 1) * 128],
                                hT[:, ik, t0 + k : t0 + k + TTILE],
                                start=(acc == 0),
                                stop=(acc == n_acc - 1),
                            )
                            acc += 1
                    nc.scalar.activation(
                        yt[:, o, :],
                        ps[:],
                        AF.Silu,
                        bias=cb_sb[:, o : o + 1],
                    )
                # Stage 3: partial out[t, :] = y^T.T @ W_out_g^T
                last_tile = b == B - 1 and tt == n_tt - 1
                for ss in range(TTILE // 128):
                    ot = opool.tile([128, H], dt.float32, tag="ot")
                    for nh in range(NH):
                        ps = ps3pool.tile([128, TTILE], dt.float32)
                        for oo in range(CT):
                            nc.tensor.matmul(
                                ps[:],
                                yt[:, oo, ss * 128 : (ss + 1) * 128],
                                wo_sb[:, oo, nh * TTILE : (nh + 1) * TTILE],
                                start=(oo == 0),
                                stop=(oo == CT - 1),
                            )
                        nc.vector.tensor_copy(
                            ot[:, nh * TTILE : (nh + 1) * TTILE], ps[:]
                        )
                        row = tg + ss * 128
                        # The final tile's stores sit on the kernel-exit
                        # critical path (one descriptor ~= one DMA engine
                        # at ~65 GB/s): split them for parallel drain.
                        n_split = 4 if last_tile else 1
                        w = TTILE // n_split
                        for sp in range(n_split):
                            col = nh * TTILE + sp * w
                            nc.sync.dma_start(
                                out[row : row + 128, col : col + w],
                                ot[:, col : col + w],
                            )

            # Stage 1 runs one time-tile ahead of stages 2/3: keeps the PE
            # stream dense and moves the cw/wo DMA deadlines out by a tile.
            for i, (b, tt) in enumerate(tiles):
                stage1(b, tt)
                if i > 0:
                    stage23(*tiles[i - 1])
            stage23(*tiles[-1])

    nc.compile()
    return nc


def _prep_inputs(x, W_in, b_in, conv_w, conv_b, W_out):
    """Host-side shard + transpose + bf16 cast. Returns in_maps for 8 cores."""
    x = np.asarray(x, dtype=np.float32)
    # x^T in [h_inner=128, h_outer, t] layout
    xr = (
        x.reshape(T, HK, 128).transpose(2, 1, 0).astype(_BF16)
    )  # [128, HK, T]
    xr = np.ascontiguousarray(xr)

    in_maps = []
    for g in range(NCORES):
        c0 = g * CG
        w_in_g = np.ascontiguousarray(
            np.asarray(W_in[c0 : c0 + CG, :])
            .reshape(CT, 128, HK, 128)
            .transpose(3, 0, 2, 1)
            .astype(_BF16)
        )  # [128, CT, HK, 128]: (hi, cc, hk, ci) = W_in[c0+cc*128+ci, hk*128+hi]
        cw_g = np.ascontiguousarray(
            np.asarray(conv_w[c0 : c0 + CG, :, :])
            .reshape(CG, CT, 128, KT)
            .transpose(2, 3, 1, 0)
            .astype(_BF16)
        )  # [128, KT, CT, CG]: (ii, k, io, o) = conv_w[c0+o, io*128+ii, k]
        wo_g = np.ascontiguousarray(
            np.asarray(W_out[:, c0 : c0 + CG])
            .reshape(H, CT, 128)
            .transpose(2, 1, 0)
            .astype(_BF16)
        )  # [128, CT, H]: (oi, oo, h) = W_out[h, c0+oo*128+oi]
        bin_g = np.ascontiguousarray(
            np.asarray(b_in[c0 : c0 + CG], dtype=np.float32).reshape(CT, 128).T
        )  # [128, CT]
        cb_g = np.ascontiguousarray(
            np.asarray(conv_b[c0 : c0 + CG], dtype=np.float32).reshape(CT, 128).T
        )
        in_maps.append(
            {
                "xT": xr,
                "w_in": w_in_g,
                "cw": cw_g,
                "wo": wo_g,
                "b_in": bin_g,
                "cb": cb_g,
            }
        )
    return in_maps


def kernel(x, W_in, b_in, conv_w, conv_b, W_out, b_out):
    global LAST_RESULTS
    from concourse import bass_utils

    if "nc" not in _CACHE:
        _CACHE["nc"] = _build_nc()
    nc = _CACHE["nc"]

    in_maps = _prep_inputs(x, W_in, b_in, conv_w, conv_b, W_out)

    res = bass_utils.run_bass_kernel_spmd(
        nc, in_maps, core_ids=list(range(NCORES))
    )
    LAST_RESULTS = res

    acc = np.array(res.results[0]["out"], dtype=np.float32, copy=True)
    for r in res.results[1:]:
        acc += r["out"]
    acc += np.asarray(b_out, dtype=np.float32)[None, :]
    return acc.reshape(B, S, H)

